# revision 21
# baseline (speedup 1.0000x reference)
"""Greedy CTC decoder on Trainium2 (Bass/Tile), sharded over 8 NeuronCores.

Input : emission [65536, 512] float32 (full, unsharded)
Output: (index [65536] int32, keep [65536] bool) matching the reference:
    index = argmax(emission, axis=-1)
    keep  = (index != prev_index) & (index != 0), prev of t=0 is a sentinel

Sharding: timestep axis T split across 8 cores (8192 rows each). Inside a
core, partition p owns the 64 consecutive timesteps p*64..p*64+63.

Device algorithm: the DVE both produces the exact per-row max (FIND needles
must be bitwise-exact) and locates it, at 1 elem/cycle/partition — so the
kernel minimizes DVE element traffic. A 3-level TENSOR_TENSOR max tree over
adjacent-column pairs (each level streams two operands through both SBUF
read ports, 1 output/cycle) compresses each 512-wide row to g3 (64 wide,
g3[i] = max of columns [8i, 8i+8)); TENSOR_REDUCE over g3 gives the exact
row max and FIND_INDEX8 scans g3 for up to 8 rows at once. Total DVE cost
is 4608 cycles per 8 rows versus 8192 for the direct reduce+find. Adjacent
pairing keeps block order = column order, so the first matching block holds
the first max occurrence; the host picks the argmax within the 8-column
block (vectorized gather + argmax) and computes the repeat-collapse mask —
O(T) postprocessing, same order as the shard-boundary exchange.
"""

import numpy as np

import concourse.bacc as bacc
import concourse.mybir as mybir
from concourse.tile import TileContext
from concourse.bass_utils import run_bass_kernel_spmd

N_CORES = 8
T_FULL = 65536
V = 512
P = 128
T_SHARD = T_FULL // N_CORES          # 8192
JPP = T_SHARD // P                   # 64 timesteps per partition
# processing units: each unit is a list of per-DMA row counts. A unit's
# rows share one SBUF tile and one TT2/TT3/reduce (fewer instructions);
# TT1 and the DMAs stay at 8-row granularity so the DVE never waits on
# more than 2 MiB of inflight data. Small units at the ends keep the
# startup and drain tails short.
UNITS = [[2], [2], [4], [8, 8], [8, 8], [8, 8], [4], [2], [2]]
CHUNKS = [sum(u) for u in UNITS]
CHUNK_STARTS = np.cumsum([0] + CHUNKS[:-1]).astype(np.int64)
MID = 40                             # mid-stream output-DMA boundary
G3 = V // 8                          # 64 g3 elements per row

_prog_cache = {}


def _build():
    nc = bacc.Bacc(None, target_bir_lowering=False)

    em_h = nc.dram_tensor("emission", [T_SHARD, V], mybir.dt.float32,
                          kind="ExternalInput")
    idx_h = nc.dram_tensor("idx_out", [T_SHARD], mybir.dt.uint32,
                           kind="ExternalOutput")

    # [T_SHARD, V] -> [P, JPP, V]: partition p holds rows p*JPP .. p*JPP+JPP-1
    em3 = em_h[:, :].rearrange("(p j) v -> p j v", p=P)
    idx_out2 = idx_h[:].rearrange("(p j) -> p j", p=P)

    with TileContext(nc) as tc:
        with (
            tc.tile_pool(name="io", bufs=3) as io_pool,
            tc.tile_pool(name="g1", bufs=2) as g1_pool,
            tc.tile_pool(name="g2", bufs=2) as g2_pool,
            tc.tile_pool(name="g3", bufs=3) as g3_pool,
            tc.tile_pool(name="mx", bufs=4) as mx_pool,
            tc.tile_pool(name="acc", bufs=1) as acc_pool,
        ):
            # raw block-index stream: batched FIND_INDEX8 over a chunk's g3
            # returns k*64 + i for row k of the chunk. The find always
            # writes 8 slots, so pad the tail; slots beyond a chunk's rows
            # are garbage that later chunks (or nothing) overwrite.
            idxr = acc_pool.tile([P, JPP + 8], mybir.dt.uint32)

            j = 0
            for c, unit in enumerate(UNITS):
                n = sum(unit)
                tile = io_pool.tile([P, n, V], mybir.dt.float32)
                off = 0
                for sub in unit:
                    nc.sync.dma_start(out=tile[:, off:off + sub, :],
                                      in_=em3[:, j + off:j + off + sub, :])
                    off += sub
                g1 = g1_pool.tile([P, n, V // 2], mybir.dt.float32)
                g2 = g2_pool.tile([P, n, V // 4], mybir.dt.float32)
                g3 = g3_pool.tile([P, n, G3], mybir.dt.float32)
                rowmax = mx_pool.tile([P, max(n, 8)], mybir.dt.float32)
                off = 0
                for sub in unit:
                    p1 = tile[:, off:off + sub, :].rearrange(
                        "p a (v w) -> p a v w", w=2)
                    nc.vector.tensor_tensor(out=g1[:, off:off + sub, :],
                                            in0=p1[:, :, :, 0],
                                            in1=p1[:, :, :, 1],
                                            op=mybir.AluOpType.max)
                    off += sub
                p2 = g1[:, :, :].rearrange("p a (v w) -> p a v w", w=2)
                nc.vector.tensor_tensor(out=g2[:, :, :], in0=p2[:, :, :, 0],
                                        in1=p2[:, :, :, 1],
                                        op=mybir.AluOpType.max)
                p3 = g2[:, :, :].rearrange("p a (v w) -> p a v w", w=2)
                nc.vector.tensor_tensor(out=g3[:, :, :], in0=p3[:, :, :, 0],
                                        in1=p3[:, :, :, 1],
                                        op=mybir.AluOpType.max)
                if n < 8:
                    # unused needle slots must hold something deterministic:
                    # their matches land in garbage idxr slots anyway
                    nc.vector.memset(rowmax[:, n:8], 0.0)
                nc.vector.tensor_reduce(out=rowmax[:, 0:n], in_=g3[:, :, :],
                                        axis=mybir.AxisListType.X,
                                        op=mybir.AluOpType.max)
                # FIND_INDEX8 takes 8 needles per scan: one per 8-row group
                for b in range(0, n, 8):
                    hi = min(b + 8, n)
                    nc.vector.max_index(
                        out=idxr[:, j + b:j + b + 8],
                        in_max=rowmax[:, b:b + 8],
                        in_values=g3[:, b:hi, :].rearrange("p a v -> p (a v)"))
                j += n
                if j == MID:
                    nc.sync.dma_start(out=idx_out2[:, 0:MID],
                                      in_=idxr[:, 0:MID])

            nc.sync.dma_start(out=idx_out2[:, MID:JPP], in_=idxr[:, MID:JPP])

    nc.compile()
    return nc


def _get_prog():
    if "nc" not in _prog_cache:
        _prog_cache["nc"] = _build()
    return _prog_cache["nc"]


# per-jj expected row-within-find-group bits (raw >> 6) for collision
# detection: each FIND_INDEX8 covers up to 8 rows from its chunk's start
_jj = np.arange(JPP)
_start_of = np.zeros(JPP, dtype=np.int64)
for _s, _n in zip(CHUNK_STARTS, CHUNKS):
    _start_of[_s:_s + _n] = _s
EXPECTED_K = ((_jj - _start_of) % 8).astype(np.uint32)


def run_sharded(emission: np.ndarray, **spmd_kwargs):
    """Run the SPMD kernel; returns (idx int32 [T], keep bool [T], results)."""
    emission = np.ascontiguousarray(np.asarray(emission, dtype=np.float32))
    assert emission.shape == (T_FULL, V), emission.shape
    nc = _get_prog()
    in_maps = [
        {"emission": np.ascontiguousarray(emission[c * T_SHARD:(c + 1) * T_SHARD])}
        for c in range(N_CORES)
    ]
    res = run_bass_kernel_spmd(nc, in_maps, list(range(N_CORES)), **spmd_kwargs)
    raw = np.concatenate([res.results[c]["idx_out"] for c in range(N_CORES)])

    # device gave the first 8-column block containing the row max; pick the
    # argmax within the block (first occurrence, matching the reference)
    t_all = np.arange(T_FULL)
    i_star = (raw & (G3 - 1)).astype(np.int64)
    block = emission[t_all[:, None], 8 * i_star[:, None] + np.arange(8)]
    idx = (8 * i_star + np.argmax(block, axis=1)).astype(np.int32)

    # cross-row bitwise-equal collisions in the batched FIND_INDEX8: the
    # needle matched in the wrong row's segment; detect via the row bits
    expected = EXPECTED_K[t_all % JPP]
    corrupt = np.nonzero((raw >> 6) != expected)[0]
    for t in corrupt:
        idx[t] = int(np.argmax(emission[t]))

    # repeat-collapse mask (the original module's blank/duplicate strip)
    keep = np.empty(T_FULL, dtype=bool)
    keep[0] = idx[0] != 0
    keep[1:] = (idx[1:] != idx[:-1]) & (idx[1:] != 0)
    return idx, keep, res


def kernel(emission: np.ndarray):
    idx, keep, _ = run_sharded(emission)
    return idx, keep
